# revision 25
# baseline (speedup 1.0000x reference)
"""LoRA linear kernel for Trainium2, SPMD across 8 NeuronCores.

Computes out = x @ W.T + bias + (x @ A.T) @ B.T * (alpha/rank) for
x:[4,2048,4096], W:[4096,4096], bias:[4096], A:[16,4096], B:[4096,16].

Sharding: data-parallel over tokens. Each core takes 1024 tokens and
computes all 4096 output features. The host pre-transposes x and pre-tiles
W so the contraction dim (in_features) lands on the SBUF partition axis and
every W DMA is a single fully-contiguous 1 MB read; each core computes
out.T for its token shard and the host transposes back.

All matmul operands are bf16 (host-side cast). PE rate for bf16 equals
fp32r (1 cycle/row) but LDWEIGHTS is half the bytes so the per-matmul
weight-load hides under the previous matmul, and the input streams halve.
PSUM accumulation stays fp32; output is stored bf16 and upcast on the
host. Rel err ~4e-3 vs the fp32 reference (gate is 2e-2).

Device-side per core:
  outT[o, t] = sum_k W.T[k, o].T @ x.T[k, t]   (32 accumulating matmuls)
             + [scale*B.T; bias].T @ [xa.T; 1] (1 matmul folds LoRA + bias)
  with xa.T = A.T.T @ x.T computed once per core on-chip.

Schedule: a k-major prologue computes xa plus the first two output groups
while x/W stream in; steady state is one output group at a time, k-inner,
with each group's finish (bias+LoRA, stop=True) emitted after the next
group's first matmuls so accumulation-group turnaround hides under live
work. DMA: x on the Sync HW queue, W on the Scalar HW queue, mid-run outs
on GpSimd's software DGE (own semaphore pool), last groups' outs on Scalar
so the final barrier never waits on a slow software-DGE drain.
"""

import sys
import types

import numpy as np

_REPO = "/opt/trn_rl_repo"
if _REPO not in sys.path:
    sys.path.insert(0, _REPO)

import ml_dtypes  # noqa: E402

import concourse.bass as bass  # noqa: E402
import concourse.mybir as mybir  # noqa: E402
import concourse.tile as tile  # noqa: E402

F32 = mybir.dt.float32
BF16 = mybir.dt.bfloat16
BF16NP = ml_dtypes.bfloat16

B_BATCH, SEQ, DIN = 4, 2048, 4096
DOUT = 4096
RANK = 16
SCALE = 1.0 / 16.0
N_CORES = 8
TOK = B_BATCH * SEQ  # 8192
TOK_C = TOK // N_CORES  # 1024 tokens per core
KC = DIN // 128  # 32 contraction chunks
NC_OUT = DOUT // 128  # 32 output-feature chunks per core
TBLK = 512  # moving free dim per matmul (one PSUM bank)
NT = TOK_C // TBLK  # 2 token blocks per core


def _install_ntff_hook():
    """Best-effort shim so trace=True yields exec_time_ns under axon."""
    try:
        import antenv.axon_hooks  # noqa: F401
        return
    except ImportError:
        pass
    try:
        from trn_agent_boot.trn_boot import _ntff_profile_via_ctypes

        hook = _ntff_profile_via_ctypes("/opt/axon/libaxon_pjrt.so")
        m = types.ModuleType("antenv.axon_hooks")
        m.get_axon_ntff_profile_hook = lambda: hook
        m.set_axon_ntff_profile_hook = lambda h: None
        sys.modules["antenv.axon_hooks"] = m
        import concourse.bass_utils as bu

        bu.upload_artifacts = lambda tmpdir: f"local:{tmpdir}"
    except Exception:
        pass


def _legalize_waits(nc, max_waits=1):
    """Walrus codegen on this toolchain rejects instructions carrying more
    than a few semaphore waits. Hoist excess waits onto NoOps inserted
    immediately before the offending instruction on the same engine."""
    n_split = 0
    for fn in nc.m.functions:
        for bb in fn.blocks:
            new_list = []
            for ins in bb.instructions:
                si = ins.sync_info
                if si is not None and si.on_wait and len(si.on_wait) > max_waits:
                    waits = list(si.on_wait)
                    while len(waits) > max_waits:
                        chunk, waits = waits[:max_waits], waits[max_waits:]
                        nop = mybir.InstNoOp(
                            name=nc.get_next_instruction_name(),
                            engine=ins.engine,
                            sync_info=mybir.SyncInfo(on_wait=chunk, on_update=[]),
                            bass_nofuse=True,
                        )
                        nc.register_instruction(nop)
                        new_list.append(nop)
                        n_split += 1
                    si.on_wait = waits
                new_list.append(ins)
            bb.instructions[:] = new_list
    return n_split


def build_program():
    nc = bass.Bass()
    # xT[k*128+p, t] = x_shard.T ; per-partition lines are 2 KB contiguous.
    xT = nc.declare_dram_parameter("xT", [DIN, TOK_C], BF16, isOutput=False)
    # Wt[n*128+p, kc*128+o] = W[n*128+o, kc*128+p]: the SBUF tile layout
    # [p, kc, o] laid out row-major, so each W chunk DMA is one contiguous
    # 1 MB read (8 KB per partition line).
    Wt = nc.declare_dram_parameter("Wt", [DOUT, DIN], BF16, isOutput=False)
    # ATs[p, kc*RANK + r] = A[r, kc*128+p]
    ATs = nc.declare_dram_parameter("ATs", [128, KC * RANK], BF16, isOutput=False)
    BTb = nc.declare_dram_parameter("BTb", [RANK + 1, DOUT], BF16, isOutput=False)
    ones = nc.declare_dram_parameter("ones", [1, TOK_C], BF16, isOutput=False)
    # Output in bf16: halves the PSUM->SBUF copy time (2x DVE rate), the out
    # DMA bytes, and the end-of-kernel DGE drain. Host upcasts to fp32.
    outT = nc.declare_dram_parameter("outT", [DOUT, TOK_C], BF16, isOutput=True)

    PRO_N = 2  # n-groups folded into the k-major prologue
    LAG = 12  # group-1 trails group-0 so its W chunk has time to land

    with tile.TileContext(nc) as tc:
        with (
            tc.tile_pool(name="xpool", bufs=KC) as xpool,
            tc.tile_pool(name="atpool", bufs=1) as atpool,
            tc.tile_pool(name="xapool", bufs=1) as xapool,
            tc.tile_pool(name="wpool", bufs=PRO_N + 1) as wpool,
            tc.tile_pool(name="btpool", bufs=1) as btpool,
            tc.tile_pool(name="opool", bufs=3) as opool,
            tc.tile_pool(name="pp", bufs=6, space="PSUM") as pp,
            tc.tile_pool(name="ppxa", bufs=2, space="PSUM") as ppxa,
        ):
            def dma_w(n, ret_dma=False):
                wt = wpool.tile([128, KC * 128], BF16, tag="wt", name=f"wt{n}")
                rows = slice(n * 128, (n + 1) * 128)
                wdma = nc.scalar.dma_start(wt[:], Wt[rows, :])
                return (wt, wdma) if ret_dma else wt

            # at's k=0 slice (4KB) goes at the head of the sync queue — the
            # very first PE op needs only it; the rest of at follows the
            # first x half. Scalar stays a pure W stream. Early DMA runs at
            # ramp-limited bandwidth, so every early KB on the critical
            # path counts.
            at = atpool.tile([128, KC * RANK], BF16, name="at")
            nc.sync.dma_start(at[:, 0:RANK], ATs[:, 0:RANK])
            xa = xapool.tile([RANK + 1, TOK_C], BF16, name="xa")

            # The x stream is the prologue's binding constraint: one HW-DGE
            # queue sustains ~180 B/ns (1.46us per 256KB chunk) while the PE
            # consumes a chunk every ~0.9us pre-LAG. Split the stream across
            # BOTH queues: even chunks on sync, odd on scalar, with W0/W1
            # interleaved on scalar where the prologue needs them.
            xt, x_dmas = [None] * KC, [None] * KC

            def dma_x(k, eng):
                xk = xpool.tile([128, TOK_C], BF16, tag="xt", name=f"x{k}")
                rows = slice(k * 128, (k + 1) * 128)
                if k < 2:
                    for t in range(NT):
                        ts = slice(t * TBLK, (t + 1) * TBLK)
                        xd = eng.dma_start(xk[:, ts], xT[rows, ts])
                else:
                    xd = eng.dma_start(xk[:], xT[rows, :])
                xt[k] = xk
                x_dmas[k] = xd

            # Early DMA bandwidth is chip-ramp limited, not queue limited:
            # keep the whole x stream on sync and the W stream on scalar so
            # neither delays the other's early chunks.
            wts = {}
            dma_x(0, nc.sync)
            nc.sync.dma_start(at[:, RANK:], ATs[:, RANK:])
            for k in range(1, KC):
                dma_x(k, nc.sync)
            # First W chunk split 4-ways and W1 halved so the prologue's
            # k-pace never outruns the (ramp-limited) W stream.
            wts[0] = wpool.tile([128, KC * 128], BF16, tag="wt", name="wt0")
            for s in range(4):
                cs = slice(s * 8 * 128, (s + 1) * 8 * 128)
                nc.scalar.dma_start(wts[0][:, cs], Wt[0:128, cs])
            wts[1] = wpool.tile([128, KC * 128], BF16, tag="wt", name="wt1")
            for s in range(2):
                cs = slice(s * 16 * 128, (s + 1) * 16 * 128)
                nc.scalar.dma_start(wts[1][:, cs], Wt[128:256, cs])
            # Single [17, DOUT] tile holding scale*B.T plus the bias row for
            # ALL output chunks — one DMA, so no per-group bt DMA or
            # semaphore wait on the finish matmuls. Row 16 of xa is a
            # ones-row that injects the bias through the same accumulating
            # matmul as the LoRA term; both are first needed by
            # finish_group(0), well after the prologue.
            btall = btpool.tile([RANK + 1, DOUT], BF16, name="btall")
            nc.scalar.dma_start(btall[:], BTb[:])
            nc.scalar.dma_start(xa[RANK : RANK + 1, :], ones[:])

            def w_ap(n, k):
                return wts[n][:, k * 128 : (k + 1) * 128]

            # Prologue: k-major sweep computing xa.T and the first PRO_N
            # output-chunk groups while x is still streaming in.
            pxa = [
                ppxa.tile([RANK, TBLK], F32, tag="pxa", name=f"pxa{t}")
                for t in range(NT)
            ]
            pros = {
                (n, t): pp.tile([128, TBLK], F32, tag="ps", name=f"ps{n}_{t}")
                for n in range(PRO_N)
                for t in range(NT)
            }

            def pro_mm(n, kk, t):
                nc.tensor.matmul(
                    pros[(n, t)][:],
                    w_ap(n, kk),
                    xt[kk][:, t * TBLK : (t + 1) * TBLK],
                    start=(kk == 0),
                    stop=False,
                )

            # pxa matmuls first within each k: they depend only on x, so the
            # in-order PE can run them while the W-dependent matmuls' chunk
            # is still in flight.
            for k in range(KC):
                for t in range(NT):
                    nc.tensor.matmul(
                        pxa[t][:],
                        at[:, k * RANK : (k + 1) * RANK],
                        xt[k][:, t * TBLK : (t + 1) * TBLK],
                        start=(k == 0),
                        stop=(k == KC - 1),
                    )
                for t in range(NT):
                    pro_mm(0, k, t)
                    if k >= LAG:
                        pro_mm(1, k - LAG, t)
            for kk in range(KC - LAG, KC):
                for t in range(NT):
                    pro_mm(1, kk, t)
            for t in range(NT):
                ts = slice(t * TBLK, (t + 1) * TBLK)
                nc.vector.tensor_copy(xa[0:RANK, ts], pxa[t][:])

            def finish_group(n, ps_map):
                ot = opool.tile([128, TOK_C], BF16, tag="ot", name=f"ot{n}")
                for t in range(NT):
                    ts = slice(t * TBLK, (t + 1) * TBLK)
                    nc.tensor.matmul(
                        ps_map[t][:],
                        btall[:, n * 128 : (n + 1) * 128],
                        xa[:, ts],
                        start=False,
                        stop=True,
                    )
                    nc.vector.tensor_copy(ot[:, ts], ps_map[t][:])
                    # Mid-run out DMAs ride gpsimd's software DGE (own
                    # semaphore pool, ~25ns issue; HW-DGE queues share 8
                    # round-robin sems that long-latency outs would clog).
                    # The last few groups go out via the scalar HW queue,
                    # which transfers faster, so the end-of-kernel barrier
                    # isn't gated on a slow software-DGE drain.
                    eng = nc.gpsimd if n < NC_OUT - 4 else nc.scalar
                    eng.dma_start(outT[n * 128 : (n + 1) * 128, ts], ot[:, ts])

            # Steady state: one output-feature chunk at a time, k-inner.
            # Each group's finish matmuls (bias+LoRA, stop=True) are emitted
            # AFTER the next group's first k so their semaphore waits and the
            # accumulation-group turnaround hide under live matmuls instead
            # of stalling the PE at the group boundary.
            from concourse.tile import add_dep_helper

            pending = [(n, {t: pros[(n, t)] for t in range(NT)}) for n in range(PRO_N)]
            for n in range(PRO_N, NC_OUT):
                wts[n], wdma = dma_w(n, ret_dma=True)
                if n == PRO_N:
                    # W2 has no tile dependency, so without this it issues at
                    # t=0 and its transfer competes with the x stream for DMA
                    # bandwidth during the prologue.
                    add_dep_helper(
                        wdma.ins,
                        x_dmas[24].ins,
                        reason="hold first steady W chunk behind the x stream",
                    )
                ps_map = {}
                for t in range(NT):
                    ps = pp.tile([128, TBLK], F32, tag="ps", name=f"ps{n}_{t}")
                    ps_map[t] = ps
                    nc.tensor.matmul(
                        ps[:],
                        w_ap(n, 0),
                        xt[0][:, t * TBLK : (t + 1) * TBLK],
                        start=True,
                        stop=False,
                    )
                for pn, pm in pending:
                    finish_group(pn, pm)
                pending = [(n, ps_map)]
                for t in range(NT):
                    for k in range(1, KC):
                        nc.tensor.matmul(
                            ps_map[t][:],
                            w_ap(n, k),
                            xt[k][:, t * TBLK : (t + 1) * TBLK],
                            start=False,
                            stop=False,
                        )
            for pn, pm in pending:
                finish_group(pn, pm)

    _legalize_waits(nc)
    return nc


_PROGRAM = None


def _get_program():
    global _PROGRAM
    if _PROGRAM is None:
        _PROGRAM = build_program()
    return _PROGRAM


def prepare_in_maps(x, W, bias, A, B):
    x = np.ascontiguousarray(np.asarray(x, dtype=np.float32))
    W = np.asarray(W, dtype=np.float32)
    bias = np.asarray(bias, dtype=np.float32)
    A = np.asarray(A, dtype=np.float32)
    B = np.asarray(B, dtype=np.float32)

    xf = x.reshape(TOK, DIN)
    # Wt[n, p, kc, o] = W[n*128+o, kc*128+p]
    Wt = np.ascontiguousarray(
        W.astype(BF16NP).reshape(NC_OUT, 128, KC, 128).transpose(0, 3, 2, 1)
    ).reshape(DOUT, DIN)
    # ATs[p, kc, r] = A[r, kc*128+p]
    ATs = np.ascontiguousarray(
        A.astype(BF16NP).reshape(RANK, KC, 128).transpose(2, 1, 0)
    ).reshape(128, KC * RANK)
    BTb = np.concatenate(
        [B.T * np.float32(SCALE), bias[None, :]], axis=0
    ).astype(BF16NP)
    ones_row = np.ones((1, TOK_C), dtype=BF16NP)
    in_maps = []
    for c in range(N_CORES):
        xT_c = np.ascontiguousarray(
            xf[c * TOK_C : (c + 1) * TOK_C, :].T.astype(BF16NP, order="C")
        )
        in_maps.append(
            {
                "xT": xT_c,
                "Wt": Wt,
                "ATs": ATs,
                "BTb": BTb,
                "ones": ones_row,
            }
        )
    return in_maps


def run(x, W, bias, A, B, trace=False):
    """Returns (out [4,2048,4096], BassKernelResults)."""
    _install_ntff_hook()
    from concourse.bass_utils import run_bass_kernel_spmd

    nc = _get_program()
    in_maps = prepare_in_maps(x, W, bias, A, B)
    res = run_bass_kernel_spmd(
        nc, in_maps, core_ids=list(range(N_CORES)), trace=trace
    )
    shards = [
        res.results[c]["outT"].astype(np.float32).T for c in range(N_CORES)
    ]
    out = np.concatenate(shards, axis=0).reshape(B_BATCH, SEQ, DOUT)
    return np.ascontiguousarray(out), res


def kernel(x, W, bias, A, B):
    out, _ = run(x, W, bias, A, B, trace=False)
    return out


if __name__ == "__main__":
    rng = np.random.default_rng(0)
    x = rng.standard_normal((B_BATCH, SEQ, DIN), dtype=np.float32)
    W = rng.standard_normal((DOUT, DIN), dtype=np.float32) * 0.02
    bias = rng.standard_normal(DOUT, dtype=np.float32) * 0.02
    A = rng.standard_normal((RANK, DIN), dtype=np.float32) / RANK
    Bm = rng.standard_normal((DOUT, RANK), dtype=np.float32) * 0.02
    out, res = run(x, W, bias, A, Bm, trace=True)
    ref = x.reshape(TOK, DIN) @ W.T + bias + (x.reshape(TOK, DIN) @ A.T) @ Bm.T * SCALE
    ref = ref.reshape(B_BATCH, SEQ, DOUT)
    err = np.abs(out - ref).max() / np.abs(ref).max()
    print("rel err:", err)
    print("exec_time_ns:", res.exec_time_ns)
